# revision 8
# baseline (speedup 1.0000x reference)
"""Trainium2 8-core causal single-head attention.

Problem: x[4,4096,768] @ Wq/Wk/Wv[768,64] -> causal softmax attention -> out[4,4096,64].

Sharding: 8 cores = 4 batches x 2 query-interleave groups. Core c handles
batch b=c//2, parity h=c%2: local q-tile i (16 tiles of 128 rows) maps to
global q-tile g=2i+h. Both cores of a batch compute full-context K/V
projections locally (no collectives). Causal block structure is
SPMD-uniform: local q-tile i attends kv-tiles 0..2i+1, with the top two
kv tiles masked by per-core data masks (h=0: [tri, zero]; h=1: [ones, tri]).

On-chip layout: everything transposed. Host passes xT (so DMA is contiguous
and projections contract e on partitions). Scores are computed as
S^T[kv_p, q_f] = kT_tile.T @ qT so the exp output PT is directly the moving
operand of PV: outT[d1, q] += v1[kv,65].T @ PT[kv, q], with v1 = [v | ones]
so row 64 of outT accumulates the softmax denominator. No max subtraction
(scores ~N(0,1), |s|<~6) and no on-chip reductions at all. Host divides by
the denominator and scatters tiles back.

Scheduling: phases are interleaved (proj j=0..3, attention q-half 0,
proj j=4..7, attention q-half 1) and a warmup matmul burst covers the input
DMA so the PE HAM clock-gate ramps to 2.4 GHz and never sees an idle window.
"""

import sys

sys.path.insert(0, "/opt/trn_rl_repo")

from contextlib import ExitStack

import numpy as np
import ml_dtypes

B, T, E, D = 4, 4096, 768, 64
P = 128
TQ = T // 2          # queries per core
NQT = TQ // P        # 16 local q tiles
NKV = T // P         # 32 kv tiles
EC = E // P          # 6 contraction chunks
BF16 = ml_dtypes.bfloat16
N_WARMUP = 26        # dummy matmuls to ramp the PE HAM clock during input DMA

_CACHE = {}


def _build_bass():
    import concourse.bacc as bacc
    import concourse.mybir as mybir
    import concourse.tile as tile

    nc = bacc.Bacc("TRN2", target_bir_lowering=False)
    f32 = mybir.dt.float32
    bf16 = mybir.dt.bfloat16

    xkv_d = nc.dram_tensor("xkv", (E, T), bf16, kind="ExternalInput")
    xq_d = nc.dram_tensor("xq", (E, TQ), bf16, kind="ExternalInput")
    wq_d = nc.dram_tensor("wq", (E, D), bf16, kind="ExternalInput")
    wkv_d = nc.dram_tensor("wkv", (E, 2 * D), bf16, kind="ExternalInput")
    mprev_d = nc.dram_tensor("mask_prev", (P, P), bf16, kind="ExternalInput")
    mlast_d = nc.dram_tensor("mask_last", (P, P), bf16, kind="ExternalInput")
    ident_d = nc.dram_tensor("ident", (P, D), bf16, kind="ExternalInput")
    out_d = nc.dram_tensor("out", (D + 1, TQ), f32, kind="ExternalOutput")

    with ExitStack() as ctx:
        tc = ctx.enter_context(tile.TileContext(nc))
        const = ctx.enter_context(tc.tile_pool(name="const", bufs=1))
        xpool = ctx.enter_context(tc.tile_pool(name="x", bufs=1))
        spool = ctx.enter_context(tc.tile_pool(name="sb", bufs=1))
        ptpool = ctx.enter_context(tc.tile_pool(name="pt", bufs=3))
        obpool = ctx.enter_context(tc.tile_pool(name="ob", bufs=2))
        psh = ctx.enter_context(tc.tile_pool(name="psh", bufs=3, space="PSUM"))
        pout = ctx.enter_context(tc.tile_pool(name="pout", bufs=1, space="PSUM"))

        # ---- PE warmup: keep TensorE busy through the input-DMA phase so the
        # HAM clock gate ramps to 2.4 GHz before real matmuls start ----
        scratch = const.tile([P, 512], bf16)
        for wi in range(N_WARMUP):
            pw = psh.tile([P, 1024], f32, tag="pp")
            nc.tensor.matmul(
                pw[:, 0:512], lhsT=scratch[:, 0:P], rhs=scratch[:],
                start=True, stop=True,
            )
            if wi == N_WARMUP - 1:
                # consume the result so DCE keeps the warmup chain
                nc.vector.tensor_copy(scratch[0:1, 0:1], pw[0:1, 0:1])

        # ---- input DMA: weights/masks first (tiny, unblock first matmuls),
        # then xq, then xkv ----
        wq_t = const.tile([P, EC * D], bf16)
        nc.sync.dma_start(
            out=wq_t.rearrange("p (ec d) -> p ec d", d=D),
            in_=wq_d.rearrange("(ec p) d -> p ec d", p=P),
        )
        wkv_t = const.tile([P, EC * 2 * D], bf16)
        nc.sync.dma_start(
            out=wkv_t.rearrange("p (ec d) -> p ec d", d=2 * D),
            in_=wkv_d.rearrange("(ec p) d -> p ec d", p=P),
        )
        mprev_t = const.tile([P, P], bf16)
        nc.sync.dma_start(out=mprev_t[:], in_=mprev_d[:])
        mlast_t = const.tile([P, P], bf16)
        nc.sync.dma_start(out=mlast_t[:], in_=mlast_d[:])
        ident_t = const.tile([P, D], bf16)
        nc.sync.dma_start(out=ident_t[:], in_=ident_d[:])
        xq_t = xpool.tile([P, EC * TQ], bf16)
        xkv_t = xpool.tile([P, EC * T], bf16)

        def dma_xq(j, nsplit=1):
            # one dma_start per (ec, col-piece): spreads a chunk across many
            # DMA queues so the critical first chunks land fast
            w = 512 // nsplit
            for ec in range(EC):
                for s in range(nsplit):
                    c0 = j * 512 + s * w
                    nc.sync.dma_start(
                        out=xq_t[:, ec * TQ + c0: ec * TQ + c0 + w],
                        in_=xq_d[ec * P:(ec + 1) * P, c0: c0 + w],
                    )

        def dma_xkv(j, nsplit=1):
            w = 512 // nsplit
            for ec in range(EC):
                for s in range(nsplit):
                    c0 = j * 512 + s * w
                    nc.sync.dma_start(
                        out=xkv_t[:, ec * T + c0: ec * T + c0 + w],
                        in_=xkv_d[ec * P:(ec + 1) * P, c0: c0 + w],
                    )

        dma_xq(0, nsplit=2)
        dma_xkv(0, nsplit=2)
        dma_xq(1)
        dma_xkv(1)
        dma_xq(2)
        dma_xq(3)
        for j in range(2, T // 512):
            dma_xkv(j)

        qT_t = spool.tile([D, TQ], bf16)
        kvT_t = spool.tile([P, T], bf16)
        v1_t = spool.tile([P, NKV * (D + 1)], bf16)
        nc.vector.memset(v1_t[:], 1.0)

        def qt_proj(j):
            ps = psh.tile([P, 1024], f32, tag="pp", name=f"psq{j}")
            for ec in range(EC):
                nc.tensor.matmul(
                    ps[0:D, 0:512],
                    lhsT=wq_t[:, ec * D:(ec + 1) * D],
                    rhs=xq_t[:, ec * TQ + j * 512: ec * TQ + (j + 1) * 512],
                    start=(ec == 0),
                    stop=(ec == EC - 1),
                )
            nc.vector.tensor_copy(qT_t[:, j * 512:(j + 1) * 512], ps[0:D, 0:512])

        def kv_proj_mm(j):
            # kT/vT columns j*512..(j+1)*512
            ps = psh.tile([P, 1024], f32, tag="pp", name=f"pskv{j}")
            for ec in range(EC):
                nc.tensor.matmul(
                    ps[:, 0:512],
                    lhsT=wkv_t[:, ec * 2 * D:(ec + 1) * 2 * D],
                    rhs=xkv_t[:, ec * T + j * 512: ec * T + (j + 1) * 512],
                    start=(ec == 0),
                    stop=(ec == EC - 1),
                )
            nc.vector.tensor_copy(kvT_t[:, j * 512:(j + 1) * 512], ps[:, 0:512])

        def v_transpose(j):
            # transpose the 4 v-tiles of chunk j into v1
            # (batched: 4 PE transposes -> one DVE copy)
            pv = psh.tile([P, 1024], bf16, tag="pp", name=f"psv{j}")
            for m in range(4):
                k = 4 * j + m
                nc.tensor.transpose(
                    pv[:, m * D:(m + 1) * D],
                    in_=kvT_t[D:2 * D, k * P:(k + 1) * P],
                    identity=ident_t[D:2 * D, :],
                )
            nc.vector.tensor_copy(
                v1_t.rearrange("p (k e) -> p k e", e=D + 1)[:, 4 * j:4 * j + 4, 0:D],
                pv.rearrange("p (m e) -> p m e", e=D)[:, 0:4, :],
            )

        outp_tiles = {}

        def attn_group(cq, ks):
            # q columns [cq*1024, (cq+1)*1024), kv tiles ks
            lo, hi = cq * 1024, (cq + 1) * 1024
            if cq not in outp_tiles:
                outp_tiles[cq] = pout.tile(
                    [D + 1, 1024], f32, tag="out", name=f"outp{cq}"
                )
            outp = outp_tiles[cq]
            for k in ks:
                qs = (k // 2) * P
                cs = max(qs, lo)
                w = hi - cs
                sst = psh.tile([P, 1024], f32, tag="pp", name=f"sst{cq}_{k}")
                for half in range(0, w, 512):
                    hw = min(512, w - half)
                    nc.tensor.matmul(
                        sst[:, half:half + hw],
                        lhsT=kvT_t[0:D, k * P:(k + 1) * P],
                        rhs=qT_t[:, cs + half: cs + half + hw],
                        start=True,
                        stop=True,
                    )
                pt = ptpool.tile([P, 1024], bf16)
                nc.scalar.activation(
                    pt[:, 0:w], sst[:, 0:w],
                    func=mybir.ActivationFunctionType.Exp, scale=0.125,
                )
                if cs == qs:
                    m = mprev_t if (k % 2 == 0) else mlast_t
                    nc.vector.tensor_mul(pt[:, 0:P], pt[:, 0:P], m[:])
                for half in range(0, w, 512):
                    hw = min(512, w - half)
                    g512 = (cs + half) // 512
                    nc.tensor.matmul(
                        outp[:, cs + half - lo: cs + half - lo + hw],
                        lhsT=v1_t[:, k * (D + 1):(k + 1) * (D + 1)],
                        rhs=pt[:, half:half + hw],
                        start=(k == 0),
                        stop=(k == 8 * g512 + 7),
                    )
                # drain each 512-col block as soon as its accumulation closes
                if k == 8 * (2 * cq) + 7 or k == 8 * (2 * cq + 1) + 7:
                    g = 0 if k == 8 * (2 * cq) + 7 else 1
                    ob = obpool.tile([D + 1, 512], f32)
                    nc.vector.tensor_copy(ob[:], outp[:, g * 512:(g + 1) * 512])
                    nc.sync.dma_start(
                        out=out_d[:, lo + g * 512: lo + (g + 1) * 512], in_=ob[:]
                    )

        # attention(0) needs qT cols < 1024 only; qT cols >= 1024 deferred.
        # kv_proj matmuls run one group ahead so the PE never waits on the
        # PSUM->SBUF drain of the chunk it is about to consume.
        qt_proj(0)
        qt_proj(1)
        kv_proj_mm(0)
        for j in range(4):
            if j + 1 < 8:
                kv_proj_mm(j + 1)
            v_transpose(j)
            attn_group(0, range(4 * j, 4 * j + 4))
        qt_proj(2)
        qt_proj(3)
        for j in range(4, 8):
            if j + 1 < 8:
                kv_proj_mm(j + 1)
            v_transpose(j)
            attn_group(1, range(8 * (j - 4), 8 * (j - 4) + 8))

    nc.compile()
    return nc


def _shard_inputs(x, Wq, Wk, Wv):
    x = np.asarray(x, np.float32)
    wqb = np.asarray(Wq, np.float32).astype(BF16)
    wkvb = np.concatenate([np.asarray(Wk, np.float32), np.asarray(Wv, np.float32)], axis=1).astype(BF16)
    ident = np.zeros((P, D), BF16)
    ident[D:2 * D, :] = np.eye(D, dtype=BF16)
    tri = (np.arange(P)[:, None] <= np.arange(P)[None, :]).astype(BF16)
    ones = np.ones((P, P), BF16)
    zeros = np.zeros((P, P), BF16)
    qidx = {h: np.concatenate([np.arange(P) + (2 * i + h) * P for i in range(NQT)]) for h in (0, 1)}
    in_maps = []
    for c in range(8):
        b, h = c // 2, c % 2
        xT = np.ascontiguousarray(x[b].T).astype(BF16)      # [768, 4096]
        xq = np.ascontiguousarray(xT[:, qidx[h]])           # [768, 2048]
        in_maps.append({
            "xkv": xT,
            "xq": xq,
            "wq": wqb,
            "wkv": wkvb,
            "mask_prev": tri if h == 0 else ones,
            "mask_last": zeros if h == 0 else tri,
            "ident": ident,
        })
    return in_maps


def _unshard(results):
    out = np.zeros((B, T, D), np.float32)
    for c, om in enumerate(results):
        b, h = c // 2, c % 2
        o = np.asarray(om["out"], np.float32)               # [65, 2048]
        on = (o[:D] / o[D:D + 1]).T                         # [2048, 64]
        for i in range(NQT):
            out[b, (2 * i + h) * P:(2 * i + h + 1) * P] = on[i * P:(i + 1) * P]
    return out


def kernel(x, Wq, Wk, Wv):
    from concourse import bass_utils

    if "nc" not in _CACHE:
        _CACHE["nc"] = _build_bass()
    nc = _CACHE["nc"]
    in_maps = _shard_inputs(x, Wq, Wk, Wv)
    res = bass_utils.run_bass_kernel_spmd(nc, in_maps, core_ids=list(range(8)))
    _CACHE["last_result"] = res
    return _unshard(res.results)


# revision 9
# speedup vs baseline: 1.1600x; 1.1600x over previous
"""Trainium2 8-core causal single-head attention.

Problem: x[4,4096,768] @ Wq/Wk/Wv[768,64] -> causal softmax attention -> out[4,4096,64].

Sharding: 8 cores = 4 batches x 2 query-interleave groups. Core c handles
batch b=c//2, parity h=c%2: local q-tile i (16 tiles of 128 rows) maps to
global q-tile g=2i+h. Both cores of a batch compute full-context K/V
projections locally (no collectives). Causal block structure is
SPMD-uniform: local q-tile i attends kv-tiles 0..2i+1, with the top two
kv tiles masked by per-core data masks (h=0: [tri, zero]; h=1: [ones, tri]).

On-chip layout: everything transposed. Host passes xT (so DMA is contiguous
and projections contract e on partitions). Scores are computed as
S^T[kv_p, q_f] = kT_tile.T @ qT so the exp output PT is directly the moving
operand of PV: outT[d1, q] += v1[kv,65].T @ PT[kv, q], with v1 = [v | ones]
so row 64 of outT accumulates the softmax denominator. No max subtraction
(scores ~N(0,1), |s|<~6) and no on-chip reductions at all. Host divides by
the denominator and scatters tiles back.

Scheduling: phases are interleaved (proj j=0..3, attention q-half 0,
proj j=4..7, attention q-half 1) and a warmup matmul burst covers the input
DMA so the PE HAM clock-gate ramps to 2.4 GHz and never sees an idle window.
"""

import sys

sys.path.insert(0, "/opt/trn_rl_repo")

from contextlib import ExitStack

import numpy as np
import ml_dtypes

B, T, E, D = 4, 4096, 768, 64
P = 128
TQ = T // 2          # queries per core
NQT = TQ // P        # 16 local q tiles
NKV = T // P         # 32 kv tiles
EC = E // P          # 6 contraction chunks
BF16 = ml_dtypes.bfloat16
N_WARMUP = 20        # dummy matmuls to ramp the PE HAM clock during input DMA

_CACHE = {}


def _build_bass():
    import concourse.bacc as bacc
    import concourse.mybir as mybir
    import concourse.tile as tile

    nc = bacc.Bacc("TRN2", target_bir_lowering=False)
    f32 = mybir.dt.float32
    bf16 = mybir.dt.bfloat16

    xkv_d = nc.dram_tensor("xkv", (E, T), bf16, kind="ExternalInput")
    xq_d = nc.dram_tensor("xq", (E, TQ), bf16, kind="ExternalInput")
    wq_d = nc.dram_tensor("wq", (E, D), bf16, kind="ExternalInput")
    wkv_d = nc.dram_tensor("wkv", (E, 2 * D), bf16, kind="ExternalInput")
    mprev_d = nc.dram_tensor("mask_prev", (P, P), bf16, kind="ExternalInput")
    mlast_d = nc.dram_tensor("mask_last", (P, P), bf16, kind="ExternalInput")
    ident_d = nc.dram_tensor("ident", (P, D), bf16, kind="ExternalInput")
    out_d = nc.dram_tensor("out", (D + 1, TQ), f32, kind="ExternalOutput")

    with ExitStack() as ctx:
        tc = ctx.enter_context(tile.TileContext(nc))
        const = ctx.enter_context(tc.tile_pool(name="const", bufs=1))
        xpool = ctx.enter_context(tc.tile_pool(name="x", bufs=1))
        spool = ctx.enter_context(tc.tile_pool(name="sb", bufs=1))
        ptpool = ctx.enter_context(tc.tile_pool(name="pt", bufs=3))
        obpool = ctx.enter_context(tc.tile_pool(name="ob", bufs=2))
        psh = ctx.enter_context(tc.tile_pool(name="psh", bufs=3, space="PSUM"))
        pout = ctx.enter_context(tc.tile_pool(name="pout", bufs=1, space="PSUM"))

        # ---- PE warmup: keep TensorE busy through the input-DMA phase so the
        # HAM clock gate ramps to 2.4 GHz before real matmuls start ----
        scratch = const.tile([P, 512], bf16)
        for wi in range(N_WARMUP):
            pw = psh.tile([P, 1024], f32, tag="pp")
            nc.tensor.matmul(
                pw[:, 0:512], lhsT=scratch[:, 0:P], rhs=scratch[:],
                start=True, stop=True,
            )
            if wi == N_WARMUP - 1:
                # consume the result so DCE keeps the warmup chain
                nc.vector.tensor_copy(scratch[0:1, 0:1], pw[0:1, 0:1])

        # ---- input DMA: weights/masks first (tiny, unblock first matmuls),
        # then xq, then xkv ----
        wq_t = const.tile([P, EC * D], bf16)
        nc.sync.dma_start(
            out=wq_t.rearrange("p (ec d) -> p ec d", d=D),
            in_=wq_d.rearrange("(ec p) d -> p ec d", p=P),
        )
        wkv_t = const.tile([P, EC * 2 * D], bf16)
        nc.sync.dma_start(
            out=wkv_t.rearrange("p (ec d) -> p ec d", d=2 * D),
            in_=wkv_d.rearrange("(ec p) d -> p ec d", p=P),
        )
        mprev_t = const.tile([P, P], bf16)
        nc.sync.dma_start(out=mprev_t[:], in_=mprev_d[:])
        mlast_t = const.tile([P, P], bf16)
        nc.sync.dma_start(out=mlast_t[:], in_=mlast_d[:])
        ident_t = const.tile([P, D], bf16)
        nc.sync.dma_start(out=ident_t[:], in_=ident_d[:])
        xq_t = xpool.tile([P, EC * TQ], bf16)
        xkv_t = xpool.tile([P, EC * T], bf16)

        def dma_xq(j, fine=False):
            if fine:
                # critical first chunk: one start per ec so it spans 6 queues
                for ec in range(EC):
                    nc.sync.dma_start(
                        out=xq_t[:, ec * TQ + j * 512: ec * TQ + (j + 1) * 512],
                        in_=xq_d[ec * P:(ec + 1) * P, j * 512:(j + 1) * 512],
                    )
            else:
                nc.sync.dma_start(
                    out=xq_t.rearrange("p (ec t) -> p ec t", t=TQ)[:, :, j * 512:(j + 1) * 512],
                    in_=xq_d.rearrange("(ec p) t -> p ec t", p=P)[:, :, j * 512:(j + 1) * 512],
                )

        def dma_xkv(j, fine=False):
            if fine:
                for ec in range(EC):
                    nc.sync.dma_start(
                        out=xkv_t[:, ec * T + j * 512: ec * T + (j + 1) * 512],
                        in_=xkv_d[ec * P:(ec + 1) * P, j * 512:(j + 1) * 512],
                    )
            else:
                nc.sync.dma_start(
                    out=xkv_t.rearrange("p (ec t) -> p ec t", t=T)[:, :, j * 512:(j + 1) * 512],
                    in_=xkv_d.rearrange("(ec p) t -> p ec t", p=P)[:, :, j * 512:(j + 1) * 512],
                )

        dma_xq(0, fine=True)
        dma_xkv(0, fine=True)
        dma_xq(1)
        dma_xkv(1)
        dma_xq(2)
        dma_xq(3)
        for j in range(2, T // 512):
            dma_xkv(j)

        qT_t = spool.tile([D, TQ], bf16)
        kvT_t = spool.tile([P, T], bf16)
        v1_t = spool.tile([P, NKV * (D + 1)], bf16)
        nc.vector.memset(v1_t[:], 1.0)

        def qt_proj(j):
            ps = psh.tile([P, 1024], f32, tag="pp", name=f"psq{j}")
            for ec in range(EC):
                nc.tensor.matmul(
                    ps[0:D, 0:512],
                    lhsT=wq_t[:, ec * D:(ec + 1) * D],
                    rhs=xq_t[:, ec * TQ + j * 512: ec * TQ + (j + 1) * 512],
                    start=(ec == 0),
                    stop=(ec == EC - 1),
                )
            nc.vector.tensor_copy(qT_t[:, j * 512:(j + 1) * 512], ps[0:D, 0:512])

        def kv_proj_mm(j):
            # kT/vT columns j*512..(j+1)*512
            ps = psh.tile([P, 1024], f32, tag="pp", name=f"pskv{j}")
            for ec in range(EC):
                nc.tensor.matmul(
                    ps[:, 0:512],
                    lhsT=wkv_t[:, ec * 2 * D:(ec + 1) * 2 * D],
                    rhs=xkv_t[:, ec * T + j * 512: ec * T + (j + 1) * 512],
                    start=(ec == 0),
                    stop=(ec == EC - 1),
                )
            nc.vector.tensor_copy(kvT_t[:, j * 512:(j + 1) * 512], ps[:, 0:512])

        def v_transpose(j):
            # transpose the 4 v-tiles of chunk j into v1
            # (batched: 4 PE transposes -> one DVE copy)
            pv = psh.tile([P, 1024], bf16, tag="pp", name=f"psv{j}")
            for m in range(4):
                k = 4 * j + m
                nc.tensor.transpose(
                    pv[:, m * D:(m + 1) * D],
                    in_=kvT_t[D:2 * D, k * P:(k + 1) * P],
                    identity=ident_t[D:2 * D, :],
                )
            nc.vector.tensor_copy(
                v1_t.rearrange("p (k e) -> p k e", e=D + 1)[:, 4 * j:4 * j + 4, 0:D],
                pv.rearrange("p (m e) -> p m e", e=D)[:, 0:4, :],
            )

        outp_tiles = {}

        def attn_group(cq, ks):
            # q columns [cq*1024, (cq+1)*1024), kv tiles ks
            lo, hi = cq * 1024, (cq + 1) * 1024
            if cq not in outp_tiles:
                outp_tiles[cq] = pout.tile(
                    [D + 1, 1024], f32, tag="out", name=f"outp{cq}"
                )
            outp = outp_tiles[cq]
            for k in ks:
                qs = (k // 2) * P
                cs = max(qs, lo)
                w = hi - cs
                sst = psh.tile([P, 1024], f32, tag="pp", name=f"sst{cq}_{k}")
                for half in range(0, w, 512):
                    hw = min(512, w - half)
                    nc.tensor.matmul(
                        sst[:, half:half + hw],
                        lhsT=kvT_t[0:D, k * P:(k + 1) * P],
                        rhs=qT_t[:, cs + half: cs + half + hw],
                        start=True,
                        stop=True,
                    )
                pt = ptpool.tile([P, 1024], bf16)
                nc.scalar.activation(
                    pt[:, 0:w], sst[:, 0:w],
                    func=mybir.ActivationFunctionType.Exp, scale=0.125,
                )
                if cs == qs:
                    m = mprev_t if (k % 2 == 0) else mlast_t
                    nc.vector.tensor_mul(pt[:, 0:P], pt[:, 0:P], m[:])
                for half in range(0, w, 512):
                    hw = min(512, w - half)
                    g512 = (cs + half) // 512
                    nc.tensor.matmul(
                        outp[:, cs + half - lo: cs + half - lo + hw],
                        lhsT=v1_t[:, k * (D + 1):(k + 1) * (D + 1)],
                        rhs=pt[:, half:half + hw],
                        start=(k == 0),
                        stop=(k == 8 * g512 + 7),
                    )
                # drain each 512-col block as soon as its accumulation closes
                if k == 8 * (2 * cq) + 7 or k == 8 * (2 * cq + 1) + 7:
                    g = 0 if k == 8 * (2 * cq) + 7 else 1
                    ob = obpool.tile([D + 1, 512], f32)
                    nc.vector.tensor_copy(ob[:], outp[:, g * 512:(g + 1) * 512])
                    nc.sync.dma_start(
                        out=out_d[:, lo + g * 512: lo + (g + 1) * 512], in_=ob[:]
                    )

        # attention(0) needs qT cols < 1024 only; qT cols >= 1024 deferred.
        # kv_proj matmuls run one group ahead so the PE never waits on the
        # PSUM->SBUF drain of the chunk it is about to consume.
        qt_proj(0)
        qt_proj(1)
        kv_proj_mm(0)
        for j in range(4):
            if j + 1 < 8:
                kv_proj_mm(j + 1)
            v_transpose(j)
            attn_group(0, range(4 * j, 4 * j + 4))
        qt_proj(2)
        qt_proj(3)
        for j in range(4, 8):
            if j + 1 < 8:
                kv_proj_mm(j + 1)
            v_transpose(j)
            attn_group(1, range(8 * (j - 4), 8 * (j - 4) + 8))

    nc.compile()
    return nc


def _shard_inputs(x, Wq, Wk, Wv):
    x = np.asarray(x, np.float32)
    wqb = np.asarray(Wq, np.float32).astype(BF16)
    wkvb = np.concatenate([np.asarray(Wk, np.float32), np.asarray(Wv, np.float32)], axis=1).astype(BF16)
    ident = np.zeros((P, D), BF16)
    ident[D:2 * D, :] = np.eye(D, dtype=BF16)
    tri = (np.arange(P)[:, None] <= np.arange(P)[None, :]).astype(BF16)
    ones = np.ones((P, P), BF16)
    zeros = np.zeros((P, P), BF16)
    qidx = {h: np.concatenate([np.arange(P) + (2 * i + h) * P for i in range(NQT)]) for h in (0, 1)}
    in_maps = []
    for c in range(8):
        b, h = c // 2, c % 2
        xT = np.ascontiguousarray(x[b].T).astype(BF16)      # [768, 4096]
        xq = np.ascontiguousarray(xT[:, qidx[h]])           # [768, 2048]
        in_maps.append({
            "xkv": xT,
            "xq": xq,
            "wq": wqb,
            "wkv": wkvb,
            "mask_prev": tri if h == 0 else ones,
            "mask_last": zeros if h == 0 else tri,
            "ident": ident,
        })
    return in_maps


def _unshard(results):
    out = np.zeros((B, T, D), np.float32)
    for c, om in enumerate(results):
        b, h = c // 2, c % 2
        o = np.asarray(om["out"], np.float32)               # [65, 2048]
        on = (o[:D] / o[D:D + 1]).T                         # [2048, 64]
        for i in range(NQT):
            out[b, (2 * i + h) * P:(2 * i + h + 1) * P] = on[i * P:(i + 1) * P]
    return out


def kernel(x, Wq, Wk, Wv):
    from concourse import bass_utils

    if "nc" not in _CACHE:
        _CACHE["nc"] = _build_bass()
    nc = _CACHE["nc"]
    in_maps = _shard_inputs(x, Wq, Wk, Wv)
    res = bass_utils.run_bass_kernel_spmd(nc, in_maps, core_ids=list(range(8)))
    _CACHE["last_result"] = res
    return _unshard(res.results)
